# revision 1
# baseline (speedup 1.0000x reference)
"""GATv2 x2 + global-mean-pool + MLP head on 8 NeuronCores (Bass/Tile).

Sharding: destination-partitioned. Core c owns nodes [c*NPC, (c+1)*NPC);
it processes every edge whose dst is in its range, so attention softmax
segments are core-local (no cross-core softmax reductions).  Layer-1
node transforms are computed replicated; layer-2 source features are
AllGathered; mean-pool partials are AllReduced; the tiny dense head is
replicated.

|att| is folded into Wl/Wr/We on the host (channels permuted so
positive-att channels come first), so the per-edge attention logit is
    e = sum_c sign_c * leaky(t_c),  t = |att| * (xl[src]+xr[dst]+ew)
computed as two Prelu passes (the negative half uses scale=-0.2,
alpha=5, whose output is exactly -leaky(t)) + a free-dim reduce + exp.
1/|att| is folded into the next layer's weights (exact, host-side).
exp is applied without max-subtraction: logits are O(1) here, so this
is numerically identical to the reference softmax.
"""

import sys
import numpy as np
DEBUG = False
import ml_dtypes

sys.path.insert(0, "/opt/trn_rl_repo")

BF16 = ml_dtypes.bfloat16

DEFAULT_CFG = dict(
    N=50000, E=500000, G=64,
    DIN=128, ED=32, H1=256, H2=128, HD=64, OUT=8,
    NC=8, HALF=32768,
)


def _roundup(x, m):
    return (x + m - 1) // m * m


def _wrap16(idx, L):
    out = np.zeros((128, max(L // 16, 1)), np.int16)
    n = len(idx)
    if n:
        pos = np.arange(n)
        out[pos % 16, pos // 16] = idx.astype(np.int16)
    for g in range(1, 8):
        out[g * 16:(g + 1) * 16] = out[0:16]
    return out


def host_prep(inputs, cfg):
    c = dict(cfg)
    N, E, G = c["N"], c["E"], c["G"]
    DIN, ED, H1, H2 = c["DIN"], c["ED"], c["H1"], c["H2"]
    NCORE, HALF = c["NC"], c["HALF"]
    NPC = N // NCORE
    NBK = _roundup(NPC, 128) // 128
    BPC = NBK * 128
    NPAD1 = _roundup(N, 512)
    NPAD2 = NCORE * BPC

    f64 = lambda x: np.asarray(x, np.float64)
    att1, att2 = f64(inputs["att1"]), f64(inputs["att2"])
    a1 = np.maximum(np.abs(att1), 1e-12); s1 = np.where(att1 >= 0, 1.0, -1.0)
    a2 = np.maximum(np.abs(att2), 1e-12); s2 = np.where(att2 >= 0, 1.0, -1.0)
    perm1 = np.argsort(-s1, kind="stable"); P1 = int((s1 > 0).sum())
    perm2 = np.argsort(-s2, kind="stable"); P2 = int((s2 > 0).sum())
    a1p, a2p = a1[perm1], a2[perm2]

    Wl1p = (f64(inputs["Wl1"]) * a1)[:, perm1]
    Wr1p = (f64(inputs["Wr1"]) * a1)[:, perm1]
    We1p = (f64(inputs["We1"]) * a1)[:, perm1]
    bl1p = (f64(inputs["bl1"]) * a1)[perm1]
    br1p = (f64(inputs["br1"]) * a1)[perm1]
    b1p = (f64(inputs["b1"]) * a1)[perm1]

    Wl2u = f64(inputs["Wl2"])[perm1, :] / a1p[:, None]
    Wr2u = f64(inputs["Wr2"])[perm1, :] / a1p[:, None]
    Wl2pp = (Wl2u * a2)[:, perm2]
    Wr2pp = (Wr2u * a2)[:, perm2]
    We2p = (f64(inputs["We2"]) * a2)[:, perm2]
    bl2p = (f64(inputs["bl2"]) * a2)[perm2]
    br2p = (f64(inputs["br2"]) * a2)[perm2]
    b2p = (f64(inputs["b2"]) * a2)[perm2]

    Wd1u = f64(inputs["Wd1"])[perm2, :] / a2p[:, None]
    bs = f64(inputs["bn_gamma"]) / np.sqrt(f64(inputs["bn_var"]) + 1e-5)
    head_scale = bs
    head_bias = (f64(inputs["bd1"]) * bs + f64(inputs["bn_beta"])
                 - f64(inputs["bn_mean"]) * bs)

    src = np.asarray(inputs["edge_src"], np.int64)
    dst = np.asarray(inputs["edge_dst"], np.int64)
    batch = np.asarray(inputs["batch"], np.int64)
    eattr = np.asarray(inputs["edge_attr"], np.float64)

    core_of = dst // NPC
    blk_of = (dst % NPC) // 128
    dloc_of = (dst % NPC) % 128

    def layer_streams(row):
        half = (row >= HALF).astype(np.int64)
        cnt = np.zeros((NCORE, NBK, 2), np.int64)
        np.add.at(cnt, (core_of, blk_of, half), 1)
        seg = _roundup(cnt.max(axis=0), 128)           # [NBK, 2]
        seg[:, 0] = np.maximum(seg[:, 0], 128)
        offs = np.zeros((NBK, 2), np.int64)
        L = 0
        for b in range(NBK):
            for h in range(2):
                offs[b, h] = L
                L += seg[b, h]
        C = L // 128
        key = core_of * (NBK * 2) + blk_of * 2 + half
        order = np.argsort(key, kind="stable")
        ks = key[order]
        idxs = np.zeros((NCORE, 128, L // 16), np.int16)
        eT = np.zeros((NCORE, ED, L), BF16)
        eE = np.zeros((NCORE, 128, C, ED + 4), BF16)
        dstrow = np.full((NCORE, 1, L), 200.0, BF16)
        dloccol = np.full((NCORE, 128, C), 200.0, np.float32)
        bounds = np.searchsorted(ks, np.arange(NCORE * NBK * 2 + 1))
        for cr in range(NCORE):
            for b in range(NBK):
                for h in range(2):
                    k = cr * (NBK * 2) + b * 2 + h
                    m = order[bounds[k]:bounds[k + 1]]
                    n = len(m)
                    o = int(offs[b, h]); sl = int(seg[b, h])
                    if sl == 0:
                        continue
                    loc_idx = np.zeros(sl, np.int64)
                    loc_idx[:n] = row[m] - h * HALF
                    idxs[cr][:, o // 16:(o + sl) // 16] = _wrap16(loc_idx, sl)
                    if n:
                        eT[cr][:, o:o + n] = eattr[m].T.astype(BF16)
                        p = np.arange(n)
                        eE[cr][p % 128, o // 128 + p // 128, :ED] = eattr[m].astype(BF16)
                        eE[cr][p % 128, o // 128 + p // 128, ED] = BF16(1.0)
                        dstrow[cr][0, o:o + n] = dloc_of[m].astype(BF16)
                        dloccol[cr][p % 128, o // 128 + p // 128] = dloc_of[m]
        return dict(seg=seg, offs=offs, L=L, C=C, idxs=idxs, eT=eT, eE=eE,
                    dstrow=dstrow, dloccol=dloccol)

    row1 = src
    row2 = BPC * (src // NPC) + (src % NPC)
    L1s = layer_streams(row1)
    L2s = layer_streams(row2)

    cnts = np.maximum(np.bincount(batch, minlength=G).astype(np.float64), 1.0)
    PT = np.zeros((NCORE, NBK, 128, G), BF16)
    for cr in range(NCORE):
        for b in range(NBK):
            base = cr * NPC + b * 128
            nn = min(128, NPC - b * 128)
            if nn <= 0:
                continue
            gids = batch[base:base + nn]
            PT[cr, b, np.arange(nn), gids] = (1.0 / cnts[gids]).astype(BF16)

    iota_col = np.arange(128, dtype=np.float32).reshape(128, 1)
    IOTAF4 = np.tile(np.arange(128, dtype=np.float32)[None, :], (128, 4))
    IDENT = np.eye(128, dtype=BF16)
    IDENT32 = np.eye(128, dtype=np.float32)
    ones1 = np.ones((1, 128), BF16)
    ones_col = np.ones((128, 1), BF16)

    node_attr_T = np.zeros((DIN, NPAD1), BF16)
    node_attr_T[:, :N] = np.asarray(inputs["node_attr"], np.float32).T.astype(BF16)

    bcast = lambda v: np.tile(np.asarray(v, np.float32)[None, :], (128, 1)).copy()

    com = dict(
        node_attr_T=node_attr_T,
        Wl1p=Wl1p.astype(BF16), Wr1p=Wr1p.astype(BF16), We1p=We1p.astype(BF16),
        Wl2pp=Wl2pp.reshape(H1 // 128, 128, H2).transpose(1, 0, 2).reshape(128, -1).astype(BF16),
        Wr2pp=Wr2pp.reshape(H1 // 128, 128, H2).transpose(1, 0, 2).reshape(128, -1).astype(BF16),
        We2p=We2p.astype(BF16),
        bl1B=bcast(bl1p), br1B=bcast(br1p), b1B=bcast(b1p),
        bl2B=bcast(bl2p), br2B=bcast(br2p), b2B=bcast(b2p),
        a1p_col=a1p.astype(np.float32).reshape(H1 // 128, 128).T.copy(),
        Wd1u=Wd1u.astype(np.float32),
        head_scale=head_scale.astype(np.float32).reshape(-1, 1),
        head_bias=head_bias.astype(np.float32).reshape(-1, 1),
        Wd2=np.asarray(inputs["Wd2"], np.float32),
        bd2=np.asarray(inputs["bd2"], np.float32).reshape(-1, 1),
        iota_col=iota_col, IOTAF4=IOTAF4, IDENT=IDENT, IDENT32=IDENT32,
        ones1=ones1, ones_col=ones_col,
    )
    percore = []
    for cr in range(NCORE):
        percore.append(dict(
            idxs1=L1s["idxs"][cr], eT1=L1s["eT"][cr], eE1=L1s["eE"][cr],
            dstrow1=L1s["dstrow"][cr], dloccol1=L1s["dloccol"][cr],
            idxs2=L2s["idxs"][cr], eT2=L2s["eT"][cr], eE2=L2s["eE"][cr],
            dstrow2=L2s["dstrow"][cr], dloccol2=L2s["dloccol"][cr],
            PT=PT[cr],
        ))
    meta = dict(cfg=c, NPC=NPC, NBK=NBK, BPC=BPC, NPAD1=NPAD1, NPAD2=NPAD2,
                P1=P1, P2=P2, L1=L1s, L2=L2s)
    return com, percore, meta


def build_program(meta, com, pc0):
    import concourse.bass as bass
    import concourse.tile as tile
    from concourse import bacc, mybir
    from concourse import library_config

    c = meta["cfg"]
    G, H1, H2, OUT = c["G"], c["H1"], c["H2"], c["OUT"]
    NCORE = c["NC"]
    BPC = meta["BPC"]
    NPAD1, NPAD2 = meta["NPAD1"], meta["NPAD2"]
    dt = mybir.dt

    nc = bacc.Bacc("TRN2", target_bir_lowering=False, debug=False,
                   num_devices=NCORE)

    dmap = {np.dtype(np.float32): dt.float32, np.dtype(BF16): dt.bfloat16,
            np.dtype(np.int16): dt.int16}
    I = {}
    for d in (com, pc0):
        for k, a in d.items():
            I[k] = nc.dram_tensor(k, list(a.shape), dmap[a.dtype],
                                  kind="ExternalInput")

    out_t = nc.dram_tensor("out", [OUT, G], dt.float32, kind="ExternalOutput")
    tbl1 = nc.dram_tensor("tbl1", [NPAD1, H1], dt.bfloat16)
    dbg = dict(x1dbg=nc.dram_tensor("x1dbg", [meta["BPC"], H1], dt.float32),
               dendbg=nc.dram_tensor("dendbg", [meta["NBK"], 128], dt.float32))
    ag2_in = nc.dram_tensor("ag2_in", [BPC, H2], dt.bfloat16)
    tbl2 = nc.dram_tensor("tbl2", [NPAD2, H2], dt.bfloat16, addr_space="Shared")
    pool_in = nc.dram_tensor("pool_in", [G, H2], dt.float32)
    pool_out = nc.dram_tensor("pool_out", [G, H2], dt.float32, addr_space="Shared")

    with tile.TileContext(nc) as tc:
        _body(nc, tc, I, out_t, tbl1, ag2_in, tbl2, pool_in, pool_out,
              meta, bass, tile, mybir, library_config, dbg)
    nc.compile()
    return nc


def _body(nc, tc, I, out_t, tbl1, ag2_in, tbl2, pool_in, pool_out,
          meta, bass, tile, mybir, library_config, dbg=None):
    from contextlib import ExitStack

    c = meta["cfg"]
    G = c["G"]
    DIN, ED, H1, H2, HD, OUT = c["DIN"], c["ED"], c["H1"], c["H2"], c["HD"], c["OUT"]
    NCORE, HALF = c["NC"], c["HALF"]
    NPC, NBK, BPC = meta["NPC"], meta["NBK"], meta["BPC"]
    NPAD1, NPAD2 = meta["NPAD1"], meta["NPAD2"]
    P1, P2 = meta["P1"], meta["P2"]
    AF = mybir.ActivationFunctionType
    dt = mybir.dt
    Alu = mybir.AluOpType
    ds = bass.ds

    nc.gpsimd.load_library(library_config.mlp)
    pid = nc.partition_id()

    ctx = ExitStack()
    with ctx:
        consts = ctx.enter_context(tc.tile_pool(name="consts", bufs=1))

        def cload(name):
            a = I[name]
            t = consts.tile(list(a.shape), a.dtype, tag=name)
            nc.sync.dma_start(t[:], a[:])
            return t

        iota_col = cload("iota_col")
        IOTAF4 = cload("IOTAF4")
        IDENT = cload("IDENT")
        IDENT32 = cload("IDENT32")
        ones1 = cload("ones1")
        ones_col = cload("ones_col")
        Wl1p = cload("Wl1p"); Wr1p = cload("Wr1p"); We1p = cload("We1p")
        Wl2pp = cload("Wl2pp"); Wr2pp = cload("Wr2pp"); We2p = cload("We2p")
        bl1B = cload("bl1B"); br1B = cload("br1B"); b1B = cload("b1B")
        bl2B = cload("bl2B"); br2B = cload("br2B"); b2B = cload("b2B")
        a1p_col = cload("a1p_col")

        res = ctx.enter_context(tc.tile_pool(name="res", bufs=1))
        xr1_nm = res.tile([128, NBK, H1], dt.bfloat16, tag="xr1")
        x1_T = res.tile([128, H1 // 128, BPC], dt.bfloat16, tag="x1T")
        xr2_nm = res.tile([128, NBK, H2], dt.bfloat16, tag="xr2")

        # ---------------- phase 1: xl1 table (replicated) + xr1 (own) --
        with tc.tile_pool(name="p1sb", bufs=4) as p1sb, \
             tc.tile_pool(name="p1ps", bufs=3, space="PSUM") as p1ps:
            NT1 = NPAD1 // 512
            for t in range(NT1):
                rhs = p1sb.tile([DIN, 512], dt.bfloat16, tag="nat")
                nc.sync.dma_start(rhs[:], I["node_attr_T"][:, t * 512:(t + 1) * 512])
                for q in range(4):
                    ps = p1ps.tile([128, H1], dt.float32, tag="xlps")
                    nc.tensor.matmul(ps[:], rhs[:, q * 128:(q + 1) * 128], Wl1p[:],
                                     start=True, stop=True)
                    sb = p1sb.tile([128, H1], dt.bfloat16, tag="xlsb")
                    nc.vector.tensor_tensor(sb[:], ps[:], bl1B[:], op=Alu.add)
                    r0 = t * 512 + q * 128
                    nc.sync.dma_start(tbl1[r0:r0 + 128, :], sb[:])
            for b in range(NBK):
                rhs = p1sb.tile([DIN, 128], dt.bfloat16, tag="natr")
                nc.sync.dma_start(rhs[:], I["node_attr_T"][:, ds(pid * NPC + b * 128, 128)])
                ps = p1ps.tile([128, H1], dt.float32, tag="xlps")
                nc.tensor.matmul(ps[:], rhs[:], Wr1p[:], start=True, stop=True)
                nc.vector.tensor_tensor(xr1_nm[:, b, :], ps[:], br1B[:], op=Alu.add)

        # ---------------- shared edge phase ----------------------------
        def edge_phase(lay, pools, pool_ps=None, PT_sb=None):
            H = H1 if lay == 1 else H2
            Ppos = P1 if lay == 1 else P2
            We = We1p if lay == 1 else We2p
            xr_nm = xr1_nm if lay == 1 else xr2_nm
            bB = b1B if lay == 1 else b2B
            tbl = tbl1 if lay == 1 else tbl2
            rows = NPAD1 if lay == 1 else NPAD2
            sfx = str(lay)
            Ls = meta["L" + sfx]
            seg, offs = Ls["seg"], Ls["offs"]
            self_base = pid * (NPC if lay == 1 else BPC)
            sb, sbg, ps_s, ps_agg, ps_sm, ps_db = pools
            tlo = tbl[0:min(HALF, rows), :]
            thi = tbl[HALF:rows, :] if rows > HALF else None
            L = int(Ls["L"]); C = int(Ls["C"])
            pre = tc.alloc_tile_pool(name="pre" + sfx, bufs=1)
            idx_all = pre.tile([128, L // 16], dt.int16, tag="idxall")
            nc.sync.dma_start(idx_all[:], I["idxs" + sfx][:])
            eE_all = pre.tile([128, C, ED + 4], dt.bfloat16, tag="eEall")
            nc.scalar.dma_start(eE_all[:], I["eE" + sfx][:])
            dlc_all = pre.tile([128, C], dt.float32, tag="dlcall")
            nc.scalar.dma_start(dlc_all[:], I["dloccol" + sfx][:])

            for b in range(NBK):
                agg = ps_agg.tile([128, H + 4], dt.float32, tag="agg")
                laden = ps_sm.tile([128, ED + 4], dt.float32, tag="sm", name="laden")
                xlw = sbg.tile([128, H], dt.bfloat16, tag="xlw")
                nc.sync.dma_start(xlw[:], tbl[ds(self_base + b * 128, 128), :])
                first = True
                nreal = int(seg[b, 0] + seg[b, 1]) // 128
                cidx = 0
                for h in range(2):
                    sl = int(seg[b, h]); o = int(offs[b, h])
                    if sl == 0:
                        continue
                    xlg = sbg.tile([128, sl // 128, H], dt.bfloat16, tag="xlg")
                    nc.gpsimd.dma_gather(xlg[:], thi if h else tlo,
                                         idx_all[:, o // 16:(o + sl) // 16], sl, sl, H)
                    eTs = sb.tile([32, ((sl + 511) // 512) * 512], dt.bfloat16, tag="eT")
                    nc.sync.dma_start(eTs[:, :sl], I["eT" + sfx][:, o:o + sl])
                    drows = sb.tile([1, ((sl + 511) // 512) * 512], dt.bfloat16, tag="drow")
                    nc.sync.dma_start(drows[:, :sl], I["dstrow" + sfx][:, o:o + sl])
                    for po in range(0, sl, 512):
                        pl = min(512, sl - po)
                        nch = pl // 128
                        jj0 = (o + po) // 128
                        drow = drows[:, po:po + 512]
                        eTt = eTs[:, po:po + 512]
                        eEt = eE_all[:, jj0:jj0 + nch, :]
                        dlc = dlc_all[:, jj0:jj0 + nch]

                        dstB = ps_db.tile([128, 512], dt.float32, tag="dstB")
                        nc.tensor.matmul(dstB[:, :pl], ones1[:], drow[:, :pl],
                                         start=True, stop=True)
                        M = sb.tile([128, 512], dt.bfloat16, tag="M")
                        nc.vector.tensor_scalar(M[:, :pl], dstB[:, :pl], iota_col[:],
                                                None, op0=Alu.is_equal)
                        s4 = ps_s.tile([128, 4, H], dt.float32, tag="s4")
                        rpc = max(1, 2048 // (H * 4))  # chunks per psum zero-region
                        for j in range(nch):
                            cs = xlg[:, po // 128 + j, :]
                            nc.tensor.matmul(s4[:, j, :],
                                             eTt[:, j * 128:(j + 1) * 128], We[:],
                                             start=(j % rpc == 0), stop=False)
                            nc.tensor.matmul(s4[:, j, :],
                                             M[:, j * 128:(j + 1) * 128], xr_nm[:, b, :],
                                             start=False, stop=False)
                            nc.tensor.matmul(s4[:, j, :], IDENT[:], cs,
                                             start=False,
                                             stop=(j % rpc == rpc - 1 or j == nch - 1))
                        ls4 = sb.tile([128, 4, H], dt.bfloat16, tag="ls4")
                        if Ppos > 0:
                            nc.scalar.activation(ls4[:, :nch, 0:Ppos], s4[:, :nch, 0:Ppos],
                                                 AF.Prelu, alpha=0.2)
                        if Ppos < H:
                            nc.scalar.activation(ls4[:, :nch, Ppos:H], s4[:, :nch, Ppos:H],
                                                 AF.Prelu, scale=-0.2, alpha=5.0)
                        e4 = sb.tile([128, 4], dt.float32, tag="e4")
                        nc.vector.reduce_sum(e4[:, :nch], ls4[:, :nch, :],
                                             axis=mybir.AxisListType.X)
                        w4 = sb.tile([128, 4], dt.float32, tag="w4")
                        nc.scalar.activation(w4[:, :nch], e4[:, :nch], AF.Exp)
                        MT = sb.tile([128, 4, 128], dt.bfloat16, tag="MT")
                        nc.vector.tensor_tensor(
                            MT[:, :nch, :],
                            IOTAF4[:].rearrange("p (a b) -> p a b", b=128)[:, :nch, :],
                            dlc.to_broadcast((128, nch, 128)),
                            op=Alu.is_equal)
                        MwT = sb.tile([128, 4, 128], dt.bfloat16, tag="MwT")
                        nc.vector.tensor_tensor(
                            MwT[:, :nch, :], MT[:, :nch, :],
                            w4[:, :nch].to_broadcast((128, nch, 128)),
                            op=Alu.mult)
                        for j in range(nch):
                            cs = xlg[:, po // 128 + j, :]
                            nc.tensor.matmul(agg[:, 0:H], MwT[:, j, :], cs,
                                             start=first, stop=False)
                            nc.tensor.matmul(agg[:, H:H + 1], MwT[:, j, :], ones_col[:],
                                             start=False, stop=False)
                            nc.tensor.matmul(laden[:, 0:ED + 2], MT[:, j, :],
                                             eE_all[:, jj0 + j, 0:ED + 2],
                                             start=(cidx == 0), stop=(cidx == nreal - 1))
                            first = False
                            cidx += 1
                # loop_attr finalize
                deg = sb.tile([128, 1], dt.float32, tag="deg")
                nc.vector.tensor_scalar(deg[:], laden[:, ED:ED + 1], 1.0, None, op0=Alu.max)
                rdeg = sb.tile([128, 1], dt.float32, tag="rdeg")
                nc.vector.reciprocal(rdeg[:], deg[:])
                la_sb = sb.tile([128, ED], dt.bfloat16, tag="lasb")
                nc.vector.tensor_scalar(la_sb[:], laden[:, 0:ED], rdeg[:], None, op0=Alu.mult)
                laT_ps = ps_sm.tile([ED, 128], dt.bfloat16, tag="sm")
                nc.tensor.transpose(laT_ps[:], la_sb[:], IDENT[:])
                laT = sb.tile([ED, 128], dt.bfloat16, tag="laTsb")
                nc.scalar.copy(laT[:], laT_ps[:])
                # self chunk
                s_s = ps_s.tile([128, 4, H], dt.float32, tag="s4")
                nc.tensor.matmul(s_s[:, 0, :], laT[:], We[:], start=True, stop=False)
                nc.tensor.matmul(s_s[:, 0, :], IDENT[:], xr_nm[:, b, :], start=False, stop=False)
                nc.tensor.matmul(s_s[:, 0, :], IDENT[:], xlw[:], start=False, stop=True)
                ls_s = sb.tile([128, 4, H], dt.bfloat16, tag="ls4")
                if Ppos > 0:
                    nc.scalar.activation(ls_s[:, 0, 0:Ppos], s_s[:, 0, 0:Ppos],
                                         AF.Prelu, alpha=0.2)
                if Ppos < H:
                    nc.scalar.activation(ls_s[:, 0, Ppos:H], s_s[:, 0, Ppos:H],
                                         AF.Prelu, scale=-0.2, alpha=5.0)
                es = sb.tile([128, 1], dt.float32, tag="es")
                nc.vector.reduce_sum(es[:], ls_s[:, 0:1, :], axis=mybir.AxisListType.X)
                ws = sb.tile([128, 1], dt.float32, tag="ws")
                nc.scalar.activation(ws[:], es[:], AF.Exp)
                diagw = sb.tile([128, 128], dt.bfloat16, tag="diagw")
                nc.vector.tensor_scalar(diagw[:], IDENT[:], ws[:], None, op0=Alu.mult)
                nc.tensor.matmul(agg[:, 0:H], diagw[:], xlw[:], start=False, stop=False)
                nc.tensor.matmul(agg[:, H:H + 1], diagw[:], ones_col[:], start=False, stop=True)
                # finalize block: x = relu(agg/den + b)
                rden = sb.tile([128, 1], dt.float32, tag="rden")
                nc.vector.reciprocal(rden[:], agg[:, H:H + 1])
                t1 = sb.tile([128, H], dt.float32, tag="t1")
                nc.vector.tensor_scalar(t1[:], agg[:, 0:H], rden[:], None, op0=Alu.mult)
                t2 = sb.tile([128, H], dt.float32, tag="t2")
                nc.vector.tensor_tensor(t2[:], t1[:], bB[:], op=Alu.add)
                x_nm = sb.tile([128, H], dt.bfloat16, tag="xnm")
                nc.scalar.activation(x_nm[:], t2[:], AF.Relu)
                if lay == 1 and DEBUG:
                    nc.sync.dma_start(dbg["x1dbg"][b * 128:(b + 1) * 128, :], t2[:])
                    nc.sync.dma_start(dbg["dendbg"][b, :], rden[:, 0])
                if lay == 1:
                    for hh in range(H1 // 128):
                        tp = ps_sm.tile([128, 128], dt.bfloat16, tag="sm")
                        nc.tensor.transpose(tp[:], x_nm[:, hh * 128:(hh + 1) * 128], IDENT[:])
                        nc.scalar.copy(x1_T[:, hh, b * 128:(b + 1) * 128], tp[:])
                else:
                    nc.tensor.matmul(pool_ps[:, 0:H2], PT_sb[b][:], x_nm[:],
                                     start=(b == 0), stop=(b == NBK - 1))
            pre.release()

        # layer-1 edge phase
        with ExitStack() as ctx1:
            pools = (
                ctx1.enter_context(tc.tile_pool(name="sb1", bufs=4)),
                ctx1.enter_context(tc.tile_pool(name="sbg1", bufs=4)),
                ctx1.enter_context(tc.tile_pool(name="ps_s1", bufs=2, space="PSUM")),
                ctx1.enter_context(tc.tile_pool(name="ps_agg1", bufs=2, space="PSUM")),
                ctx1.enter_context(tc.tile_pool(name="ps_sm1", bufs=1, space="PSUM")),
                ctx1.enter_context(tc.tile_pool(name="ps_db1", bufs=1, space="PSUM")),
            )
            edge_phase(1, pools)

        # ---------------- layer-2 node transforms + AllGather ----------
        with tc.tile_pool(name="p2sb", bufs=4) as p2sb, \
             tc.tile_pool(name="p2ps", bufs=4, space="PSUM") as p2ps:
            for b in range(NBK):
                ps = p2ps.tile([128, H2], dt.float32, tag="xl2ps")
                for hh in range(H1 // 128):
                    nc.tensor.matmul(ps[:], x1_T[:, hh, b * 128:(b + 1) * 128],
                                     Wl2pp[:, hh * H2:(hh + 1) * H2],
                                     start=(hh == 0), stop=(hh == H1 // 128 - 1))
                sbx = p2sb.tile([128, H2], dt.bfloat16, tag="xl2sb")
                nc.vector.tensor_tensor(sbx[:], ps[:], bl2B[:], op=Alu.add)
                nc.sync.dma_start(ag2_in[b * 128:(b + 1) * 128, :], sbx[:])
                ps2 = p2ps.tile([128, H2], dt.float32, tag="xr2ps")
                for hh in range(H1 // 128):
                    nc.tensor.matmul(ps2[:], x1_T[:, hh, b * 128:(b + 1) * 128],
                                     Wr2pp[:, hh * H2:(hh + 1) * H2],
                                     start=(hh == 0), stop=(hh == H1 // 128 - 1))
                nc.vector.tensor_tensor(xr2_nm[:, b, :], ps2[:], br2B[:], op=Alu.add)
        nc.gpsimd.collective_compute(
            "AllGather", mybir.AluOpType.bypass,
            replica_groups=[list(range(NCORE))],
            ins=[ag2_in[:]], outs=[tbl2[:]])

        # ---------------- layer-2 edge phase + pooling ------------------
        pool_pp = ctx.enter_context(tc.tile_pool(name="poolps", bufs=1, space="PSUM"))
        pool_ps = pool_pp.tile([G, H2 + 4], dt.float32, tag="pool")
        pt_pool = ctx.enter_context(tc.tile_pool(name="ptsb", bufs=1))
        PT_sb = []
        for b in range(NBK):
            t = pt_pool.tile([128, G], dt.bfloat16, tag=f"pt{b}")
            nc.sync.dma_start(t[:], I["PT"][b])
            PT_sb.append(t)
        with ExitStack() as ctx2:
            pools = (
                ctx2.enter_context(tc.tile_pool(name="sb2", bufs=4)),
                ctx2.enter_context(tc.tile_pool(name="sbg2", bufs=4)),
                ctx2.enter_context(tc.tile_pool(name="ps_s2", bufs=2, space="PSUM")),
                ctx2.enter_context(tc.tile_pool(name="ps_agg2", bufs=2, space="PSUM")),
                ctx2.enter_context(tc.tile_pool(name="ps_sm2", bufs=1, space="PSUM")),
                ctx2.enter_context(tc.tile_pool(name="ps_db2", bufs=1, space="PSUM")),
            )
            edge_phase(2, pools, pool_ps=pool_ps, PT_sb=PT_sb)

        # ---------------- head -----------------------------------------
        with tc.tile_pool(name="hsb", bufs=2) as hsb, \
             tc.tile_pool(name="hps", bufs=2, space="PSUM") as hps:
            psb = hsb.tile([G, H2], dt.float32, tag="poolsb")
            nc.scalar.copy(psb[:], pool_ps[:, 0:H2])
            nc.sync.dma_start(pool_in[:], psb[:])
            nc.gpsimd.collective_compute(
                "AllReduce", mybir.AluOpType.add,
                replica_groups=[list(range(NCORE))],
                ins=[pool_in[:]], outs=[pool_out[:]])
            pooled = hsb.tile([G, H2], dt.float32, tag="pooled")
            nc.sync.dma_start(pooled[:], pool_out[:])
            pooled_T_ps = hps.tile([H2, G], dt.float32, tag="pooledT")
            nc.tensor.transpose(pooled_T_ps[:], pooled[:], IDENT32[0:G, 0:G])
            pooled_T = hsb.tile([H2, G], dt.float32, tag="pooledTsb")
            nc.scalar.copy(pooled_T[:], pooled_T_ps[:])
            Wd1sb = hsb.tile([H2, HD], dt.float32, tag="wd1")
            nc.sync.dma_start(Wd1sb[:], I["Wd1u"][:])
            h1ps = hps.tile([HD, G], dt.float32, tag="h1")
            nc.tensor.matmul(h1ps[:], Wd1sb[:], pooled_T[:], start=True, stop=True)
            hscale = hsb.tile([HD, 1], dt.float32, tag="hscale")
            nc.sync.dma_start(hscale[:], I["head_scale"][:])
            hbias = hsb.tile([HD, 1], dt.float32, tag="hbias")
            nc.sync.dma_start(hbias[:], I["head_bias"][:])
            th = hsb.tile([HD, G], dt.float32, tag="th")
            nc.scalar.activation(th[:], h1ps[:], AF.Prelu, bias=hbias[:],
                                 scale=hscale[:], alpha=0.1)
            Wd2sb = hsb.tile([HD, OUT], dt.float32, tag="wd2")
            nc.sync.dma_start(Wd2sb[:], I["Wd2"][:])
            ops = hps.tile([OUT, G], dt.float32, tag="ops")
            nc.tensor.matmul(ops[:], Wd2sb[:], th[:], start=True, stop=True)
            bd2sb = hsb.tile([OUT, 1], dt.float32, tag="bd2sb")
            nc.sync.dma_start(bd2sb[:], I["bd2"][:])
            osb = hsb.tile([OUT, G], dt.float32, tag="osb")
            nc.vector.tensor_scalar(osb[:], ops[:], bd2sb[:], None, op0=Alu.add)
            nc.sync.dma_start(out_t[:], osb[:])


def _kernel(inputs, cfg, runner=None, trace=False):
    com, percore, meta = host_prep(inputs, cfg)
    nc = build_program(meta, com, percore[0])
    in_maps = [dict(com, **pc) for pc in percore]
    if runner is None:
        from concourse.bass_utils import run_bass_kernel_spmd
        res = run_bass_kernel_spmd(nc, in_maps, list(range(cfg["NC"])), trace=trace)
        out = np.asarray(res.results[0]["out"])
        return out.T.copy().astype(np.float32), res
    return runner(nc, in_maps)


def kernel(**inputs):
    out, _ = _kernel(inputs, DEFAULT_CFG)
    return out



# revision 23
# speedup vs baseline: 1.3585x; 1.3585x over previous
"""GATv2 x2 + global-mean-pool + MLP head on 8 NeuronCores (Bass/Tile). v2

Sharding: destination-partitioned. Core c owns nodes [c*NPC, (c+1)*NPC);
it processes every edge whose dst is in its range, so attention softmax
segments are core-local.

v2 layout (vs v1):
- Layer 1 is GATHER-FREE: the host lays out a duplicated node_attr
  stream in edge order (two layouts: [DIN, e] for the score stationary
  and [e, DIN] for the aggregation stationary).  Per-edge xl1 is
  computed by the PE directly into the score PSUM (natdupT @ Wl1p), and
  aggregation uses linearity:
      sum_e alpha_e xl1[src_e] = (sum_e alpha_e nat[src_e]) @ Wl1p
  so raw attrs are aggregated (128 cols/chunk) and transformed once per
  block.  This deletes the xl1 DRAM table phase AND all layer-1
  gathers.
- Self-loops are regular stream edges in layer 1 (host computes
  loop_attr = segment-mean of edge_attr, standard PyG preprocessing).
  Layer 2 keeps a per-block self chunk fed by host loop_attr (laT2).
- One-hot dst masks M [dstslot, e] come from the host (bf16 stream);
  the weighted transpose mask MwT is built in ONE fused DVE op per
  chunk: (IOTA == dloc) * w.
- Layer-2 node transforms run per-block inside the layer-1 loop and the
  tbl2 AllGather is split into 4 chunks issued as their blocks finish,
  overlapping collectives with compute.
- Linear-bias folding: bl is not materialized in xl; the score path
  gets it via xr's bias (bl+br) and the aggregation output via the
  post-agg bias (b + bl), exact because softmax weights sum to 1.

|att| is folded into Wl/Wr/We on the host (channels permuted so
positive-att channels come first); per-edge logit = two Prelu passes +
free-dim reduce + exp (no max subtraction: logits are O(1)).
1/|att| is folded into the next layer's weights (exact, host-side).
"""

import sys
import numpy as np
import ml_dtypes

sys.path.insert(0, "/opt/trn_rl_repo")

BF16 = ml_dtypes.bfloat16
DEBUG = False

DEFAULT_CFG = dict(
    N=50000, E=500000, G=64,
    DIN=128, ED=32, H1=256, H2=128, HD=64, OUT=8,
    NC=8, HALF=32768,
)

AG_SPLIT = (13, 13, 13, 10)  # blocks per AllGather chunk (sum == NBK)


def _roundup(x, m):
    return (x + m - 1) // m * m


def _wrap16(idx, L):
    out = np.zeros((128, max(L // 16, 1)), np.int16)
    n = len(idx)
    if n:
        pos = np.arange(n)
        out[pos % 16, pos // 16] = idx.astype(np.int16)
    for g in range(1, 8):
        out[g * 16:(g + 1) * 16] = out[0:16]
    return out


def host_prep(inputs, cfg):
    c = dict(cfg)
    N, E, G = c["N"], c["E"], c["G"]
    DIN, ED, H1, H2 = c["DIN"], c["ED"], c["H1"], c["H2"]
    NCORE, HALF = c["NC"], c["HALF"]
    NPC = N // NCORE
    NBK = _roundup(NPC, 128) // 128
    BPC = NBK * 128
    NPAD2 = NCORE * BPC
    assert sum(AG_SPLIT) == NBK

    f64 = lambda x: np.asarray(x, np.float64)
    att1, att2 = f64(inputs["att1"]), f64(inputs["att2"])
    a1 = np.maximum(np.abs(att1), 1e-12); s1 = np.where(att1 >= 0, 1.0, -1.0)
    a2 = np.maximum(np.abs(att2), 1e-12); s2 = np.where(att2 >= 0, 1.0, -1.0)
    perm1 = np.argsort(-s1, kind="stable"); P1 = int((s1 > 0).sum())
    perm2 = np.argsort(-s2, kind="stable"); P2 = int((s2 > 0).sum())
    a1p, a2p = a1[perm1], a2[perm2]

    Wl1p = (f64(inputs["Wl1"]) * a1)[:, perm1]
    Wr1p = (f64(inputs["Wr1"]) * a1)[:, perm1]
    We1p = (f64(inputs["We1"]) * a1)[:, perm1]
    bl1p = (f64(inputs["bl1"]) * a1)[perm1]
    br1p = (f64(inputs["br1"]) * a1)[perm1]
    b1p = (f64(inputs["b1"]) * a1)[perm1]

    Wl2u = f64(inputs["Wl2"])[perm1, :] / a1p[:, None]
    Wr2u = f64(inputs["Wr2"])[perm1, :] / a1p[:, None]
    Wl2pp = (Wl2u * a2)[:, perm2]
    Wr2pp = (Wr2u * a2)[:, perm2]
    We2p = (f64(inputs["We2"]) * a2)[:, perm2]
    bl2p = (f64(inputs["bl2"]) * a2)[perm2]
    br2p = (f64(inputs["br2"]) * a2)[perm2]
    b2p = (f64(inputs["b2"]) * a2)[perm2]

    Wd1u = f64(inputs["Wd1"])[perm2, :] / a2p[:, None]
    bs = f64(inputs["bn_gamma"]) / np.sqrt(f64(inputs["bn_var"]) + 1e-5)
    head_scale = bs
    head_bias = (f64(inputs["bd1"]) * bs + f64(inputs["bn_beta"])
                 - f64(inputs["bn_mean"]) * bs)

    src = np.asarray(inputs["edge_src"], np.int64)
    dst = np.asarray(inputs["edge_dst"], np.int64)
    batch = np.asarray(inputs["batch"], np.int64)
    eattr = np.asarray(inputs["edge_attr"], np.float64)
    node_attr = np.asarray(inputs["node_attr"], np.float32)
    natB = node_attr.astype(BF16)           # [N, DIN]
    natTB = natB.T.copy()                   # [DIN, N]

    # ---- self-loop attrs (PyG add_self_loops fill_value='mean') -------
    deg = np.bincount(dst, minlength=N).astype(np.float64)
    la = np.zeros((N, ED), np.float64)
    np.add.at(la, dst, eattr)
    la /= np.maximum(deg, 1.0)[:, None]

    # ---- layer-1 stream: real edges + self loops, grouped by dst block
    src1 = np.concatenate([src, np.arange(N, dtype=np.int64)])
    dst1 = np.concatenate([dst, np.arange(N, dtype=np.int64)])
    ea1 = np.concatenate([eattr, la], axis=0)
    core1 = dst1 // NPC
    blk1 = (dst1 % NPC) // 128
    dloc1 = (dst1 % NPC) % 128

    cnt1 = np.zeros((NCORE, NBK), np.int64)
    np.add.at(cnt1, (core1, blk1), 1)
    seg1 = _roundup(cnt1.max(axis=0), 128)          # [NBK], same on all cores
    offs1 = np.zeros(NBK, np.int64)
    L1 = 0
    for b in range(NBK):
        offs1[b] = L1
        L1 += int(seg1[b])
    C1 = L1 // 128

    key1 = core1 * NBK + blk1
    order1 = np.argsort(key1, kind="stable")
    bounds1 = np.searchsorted(key1[order1], np.arange(NCORE * NBK + 1))

    natdupT = np.zeros((NCORE, DIN, L1), BF16)
    natdupE = np.zeros((NCORE, 128, C1, DIN), BF16)
    M1 = np.zeros((NCORE, 128, L1), BF16)
    eT1 = np.zeros((NCORE, ED, L1), BF16)
    dlc1 = np.full((NCORE, 128, C1), 200.0, np.float32)
    ea1B = ea1.astype(BF16)
    for cr in range(NCORE):
        for b in range(NBK):
            k = cr * NBK + b
            m = order1[bounds1[k]:bounds1[k + 1]]
            n = len(m)
            if n == 0:
                continue
            o = int(offs1[b])
            p = o + np.arange(n)
            natdupT[cr][:, p] = natTB[:, src1[m]]
            natdupE[cr][p % 128, p // 128, :] = natB[src1[m], :]
            M1[cr][dloc1[m], p] = BF16(1.0)
            eT1[cr][:, p] = ea1B[m].T
            dlc1[cr][p % 128, p // 128] = dloc1[m]

    # ---- tbl2 row mapping: AllGather-chunk-major --------------------
    b0s = np.concatenate([[0], np.cumsum(AG_SPLIT)])[:-1]         # chunk start blk
    rpc = np.array([blks * 128 for blks in AG_SPLIT])              # rows/core/chunk
    cbase = np.concatenate([[0], np.cumsum(NCORE * rpc)])[:-1]     # global chunk base
    chunk_of_blk = np.concatenate(
        [np.full(blks, ci) for ci, blks in enumerate(AG_SPLIT)])

    def row2_of(node):
        cr_s = node // NPC
        loc = node % NPC
        b = loc // 128
        l = loc % 128
        ci = chunk_of_blk[b]
        return cbase[ci] + cr_s * rpc[ci] + (b - b0s[ci]) * 128 + l

    # ---- layer-2 stream: real edges only, (block, half) segments ----
    row2 = row2_of(src)
    core2 = dst // NPC
    blk2 = (dst % NPC) // 128
    dloc2 = (dst % NPC) % 128
    half2 = (row2 >= HALF).astype(np.int64)

    cnt2 = np.zeros((NCORE, NBK, 2), np.int64)
    np.add.at(cnt2, (core2, blk2, half2), 1)
    seg2 = _roundup(cnt2.max(axis=0), 128)          # [NBK, 2]
    seg2[:, 0] = np.maximum(seg2[:, 0], 128)
    offs2 = np.zeros((NBK, 2), np.int64)
    L2 = 0
    for b in range(NBK):
        for h in range(2):
            offs2[b, h] = L2
            L2 += int(seg2[b, h])
    C2 = L2 // 128

    key2 = core2 * (NBK * 2) + blk2 * 2 + half2
    order2 = np.argsort(key2, kind="stable")
    bounds2 = np.searchsorted(key2[order2], np.arange(NCORE * NBK * 2 + 1))

    idxs2 = np.zeros((NCORE, 128, L2 // 16), np.int16)
    eT2 = np.zeros((NCORE, ED, L2), BF16)
    M2 = np.zeros((NCORE, 128, L2), BF16)
    dlc2 = np.full((NCORE, 128, C2), 200.0, np.float32)
    eaB = eattr.astype(BF16)
    for cr in range(NCORE):
        for b in range(NBK):
            for h in range(2):
                k = cr * (NBK * 2) + b * 2 + h
                m = order2[bounds2[k]:bounds2[k + 1]]
                n = len(m)
                o = int(offs2[b, h]); sl = int(seg2[b, h])
                if sl == 0:
                    continue
                loc_idx = np.zeros(sl, np.int64)
                loc_idx[:n] = row2[m] - h * HALF
                idxs2[cr][:, o // 16:(o + sl) // 16] = _wrap16(loc_idx, sl)
                if n:
                    p = o + np.arange(n)
                    eT2[cr][:, p] = eaB[m].T
                    M2[cr][dloc2[m], p] = BF16(1.0)
                    dlc2[cr][p % 128, p // 128] = dloc2[m]

    # layer-2 self-chunk loop attrs, laid out [ED, NBK*128] per core
    laT2 = np.zeros((NCORE, ED, NBK * 128), BF16)
    for cr in range(NCORE):
        nn_ = min(BPC, NPC)
        laT2[cr][:, :nn_] = la[cr * NPC:cr * NPC + nn_].T.astype(BF16)

    # mean-pool matrix
    cnts = np.maximum(np.bincount(batch, minlength=G).astype(np.float64), 1.0)
    PT = np.zeros((NCORE, NBK, 128, G), BF16)
    for cr in range(NCORE):
        for b in range(NBK):
            base = cr * NPC + b * 128
            nn_ = min(128, NPC - b * 128)
            if nn_ <= 0:
                continue
            gids = batch[base:base + nn_]
            PT[cr, b, np.arange(nn_), gids] = (1.0 / cnts[gids]).astype(BF16)

    IOTA1 = np.tile(np.arange(128, dtype=np.float32)[None, :], (128, 1)).copy()
    IDENT = np.eye(128, dtype=BF16)
    IDENT32 = np.eye(128, dtype=np.float32)
    ones_col = np.ones((128, 1), BF16)

    natT_own = np.zeros((NCORE, DIN, BPC), BF16)
    for cr in range(NCORE):
        nn_ = min(BPC, NPC)
        natT_own[cr][:, :nn_] = natTB[:, cr * NPC:cr * NPC + nn_]

    bcast = lambda v: np.tile(np.asarray(v, np.float32)[None, :], (128, 1)).copy()

    com = dict(
        Wl1p=Wl1p.astype(BF16), Wr1p=Wr1p.astype(BF16), We1p=We1p.astype(BF16),
        Wl2pp=Wl2pp.reshape(H1 // 128, 128, H2).transpose(1, 0, 2).reshape(128, -1).astype(BF16),
        Wr2pp=Wr2pp.reshape(H1 // 128, 128, H2).transpose(1, 0, 2).reshape(128, -1).astype(BF16),
        We2p=We2p.astype(BF16),
        xr1bB=bcast(bl1p + br1p), b1B=bcast(b1p + bl1p),
        xr2bB=bcast(bl2p + br2p), b2B=bcast(b2p + bl2p),
        Wd1u=Wd1u.astype(np.float32),
        head_scale=head_scale.astype(np.float32).reshape(-1, 1),
        head_bias=head_bias.astype(np.float32).reshape(-1, 1),
        Wd2=np.asarray(inputs["Wd2"], np.float32),
        bd2=np.asarray(inputs["bd2"], np.float32).reshape(-1, 1),
        IOTA1=IOTA1, IDENT=IDENT, IDENT32=IDENT32, ones_col=ones_col,
    )
    percore = []
    for cr in range(NCORE):
        percore.append(dict(
            natdupT=natdupT[cr], natdupE=natdupE[cr], M1=M1[cr], eT1=eT1[cr],
            dlc1=dlc1[cr],
            idxs2=idxs2[cr], eT2=eT2[cr], M2=M2[cr], dlc2=dlc2[cr],
            laT2=laT2[cr], natT_own=natT_own[cr], PT=PT[cr],
        ))
    meta = dict(cfg=c, NPC=NPC, NBK=NBK, BPC=BPC, NPAD2=NPAD2,
                P1=P1, P2=P2,
                seg1=seg1, offs1=offs1, L1=L1, C1=C1,
                seg2=seg2, offs2=offs2, L2=L2, C2=C2,
                b0s=b0s, rpc=rpc, cbase=cbase)
    return com, percore, meta


def build_program(meta, com, pc0):
    import concourse.bass as bass
    import concourse.tile as tile
    from concourse import bacc, mybir
    from concourse import library_config

    c = meta["cfg"]
    G, H2, OUT = c["G"], c["H2"], c["OUT"]
    NCORE = c["NC"]
    NPAD2 = meta["NPAD2"]
    dt = mybir.dt

    nc = bacc.Bacc("TRN2", target_bir_lowering=False, debug=False,
                   num_devices=NCORE)

    dmap = {np.dtype(np.float32): dt.float32, np.dtype(BF16): dt.bfloat16,
            np.dtype(np.int16): dt.int16}
    I = {}
    for d in (com, pc0):
        for k, a in d.items():
            I[k] = nc.dram_tensor(k, list(a.shape), dmap[a.dtype],
                                  kind="ExternalInput")

    out_t = nc.dram_tensor("out", [OUT, G], dt.float32, kind="ExternalOutput")
    dbg = None
    if DEBUG:
        BPC = meta["BPC"]; H1 = c["H1"]
        sl00 = int(meta["seg2"][0, 0])
        dbg = dict(
            x1t2=nc.dram_tensor("x1t2dbg", [BPC, H1], dt.float32,
                                kind="ExternalOutput"),
            den1=nc.dram_tensor("den1dbg", [meta["NBK"], 128], dt.float32,
                                kind="ExternalOutput"),
            xl2=nc.dram_tensor("xl2dbg", [BPC, H2], dt.float32,
                               kind="ExternalOutput"),
            xr2=nc.dram_tensor("xr2dbg", [BPC, H2], dt.float32,
                               kind="ExternalOutput"),
            xlg0=nc.dram_tensor("xlg0dbg", [128, (sl00 // 128) * H2], dt.float32,
                                kind="ExternalOutput"),
            tbl2=nc.dram_tensor("tbl2dbg", [NPAD2, H2], dt.bfloat16,
                                kind="ExternalOutput"),
        )
    ag2_in = [nc.dram_tensor(f"ag2_in_{ci}", [int(r), H2], dt.bfloat16)
              for ci, r in enumerate(meta["rpc"])]
    tbl2 = nc.dram_tensor("tbl2", [NPAD2, H2], dt.bfloat16, addr_space="Shared")
    pool_in = nc.dram_tensor("pool_in", [G, H2], dt.float32)
    pool_out = nc.dram_tensor("pool_out", [G, H2], dt.float32, addr_space="Shared")

    with tile.TileContext(nc) as tc:
        _body(nc, tc, I, out_t, ag2_in, tbl2, pool_in, pool_out,
              meta, bass, tile, mybir, library_config, dbg)
    nc.compile()
    return nc


def _body(nc, tc, I, out_t, ag2_in, tbl2, pool_in, pool_out,
          meta, bass, tile, mybir, library_config, dbg=None):
    from contextlib import ExitStack

    c = meta["cfg"]
    G = c["G"]
    DIN, ED, H1, H2, HD, OUT = c["DIN"], c["ED"], c["H1"], c["H2"], c["HD"], c["OUT"]
    NCORE, HALF = c["NC"], c["HALF"]
    NPC, NBK, BPC = meta["NPC"], meta["NBK"], meta["BPC"]
    NPAD2 = meta["NPAD2"]
    P1, P2 = meta["P1"], meta["P2"]
    seg1, offs1 = meta["seg1"], meta["offs1"]
    seg2, offs2 = meta["seg2"], meta["offs2"]
    b0s, rpc, cbase = meta["b0s"], meta["rpc"], meta["cbase"]
    chunk_of_blk = np.concatenate(
        [np.full(blks, ci) for ci, blks in enumerate(AG_SPLIT)])
    chunk_last_blk = np.cumsum(AG_SPLIT) - 1
    AF = mybir.ActivationFunctionType
    dt = mybir.dt
    Alu = mybir.AluOpType
    ds = bass.ds

    nc.gpsimd.load_library(library_config.mlp)

    ctx = ExitStack()
    with ctx:
        consts = ctx.enter_context(tc.tile_pool(name="consts", bufs=1))

        def cload(name, eng=None):
            a = I[name]
            t = consts.tile(list(a.shape), a.dtype, tag=name)
            (eng or nc.sync).dma_start(t[:], a[:])
            return t

        IOTA1 = cload("IOTA1")
        IDENT = cload("IDENT")
        IDENT32 = cload("IDENT32")
        ones_col = cload("ones_col")
        Wl1p = cload("Wl1p"); Wr1p = cload("Wr1p"); We1p = cload("We1p")
        Wl2pp = cload("Wl2pp"); Wr2pp = cload("Wr2pp"); We2p = cload("We2p")
        xr1bB = cload("xr1bB"); b1B = cload("b1B")
        xr2bB = cload("xr2bB"); b2B = cload("b2B")
        natT_own = cload("natT_own")
        laT2 = cload("laT2", nc.scalar)
        dlc1_all = cload("dlc1", nc.scalar)

        res = ctx.enter_context(tc.tile_pool(name="res", bufs=1))
        xr1_nm = res.tile([128, NBK, H1], dt.bfloat16, tag="xr1")
        xr2_nm = res.tile([128, NBK, H2], dt.bfloat16, tag="xr2")
        xl2_nm = res.tile([128, NBK, H2], dt.bfloat16, tag="xl2")

        # ---------------- xr1 per block (own nodes) --------------------
        with tc.tile_pool(name="p0ps", bufs=4, space="PSUM") as p0ps:
            for b in range(NBK):
                ps = p0ps.tile([128, H1], dt.float32, tag="xr1ps")
                nc.tensor.matmul(ps[:], natT_own[:, b * 128:(b + 1) * 128],
                                 Wr1p[:], start=True, stop=True)
                nc.vector.tensor_tensor(xr1_nm[:, b, :], ps[:], xr1bB[:], op=Alu.add)

        # ---------------- layer-1 edge phase (gather-free) -------------
        with ExitStack() as ctx1:
            sbst = ctx1.enter_context(tc.tile_pool(name="sbst", bufs=2))
            sb = ctx1.enter_context(tc.tile_pool(name="sb1", bufs=4))
            ps_s = ctx1.enter_context(tc.tile_pool(name="ps_s1", bufs=2, space="PSUM"))
            ps_ag = ctx1.enter_context(tc.tile_pool(name="ps_ag1", bufs=2, space="PSUM"))
            ps_fin = ctx1.enter_context(tc.tile_pool(name="ps_fin1", bufs=1, space="PSUM"))

            for b in range(NBK):
                BL = int(seg1[b]); o = int(offs1[b])
                nchb = BL // 128
                natT_t = sbst.tile([DIN, BL], dt.bfloat16, tag="natT")
                nc.sync.dma_start(natT_t[:], I["natdupT"][:, o:o + BL])
                natE_t = sbst.tile([128, nchb, DIN], dt.bfloat16, tag="natE")
                nc.sync.dma_start(natE_t[:], I["natdupE"][:, o // 128:o // 128 + nchb, :])
                M1_t = sbst.tile([128, BL], dt.bfloat16, tag="M1")
                nc.sync.dma_start(M1_t[:], I["M1"][:, o:o + BL])
                eT1_t = sbst.tile([ED, BL], dt.bfloat16, tag="eT1")
                nc.sync.dma_start(eT1_t[:], I["eT1"][:, o:o + BL])

                agg_den = ps_ag.tile([128, 132], dt.float32, tag="aggden")
                ngrp = (nchb + 3) // 4
                for g in range(ngrp):
                    nch = min(4, nchb - 4 * g)
                    s4 = ps_s.tile([128, 4, H1], dt.float32, tag="s4")
                    for j in range(nch):
                        jj = 4 * g + j
                        es = jj * 128
                        nc.tensor.matmul(s4[:, j, :], natT_t[:, es:es + 128],
                                         Wl1p[:], start=(j % 2 == 0), stop=False)
                        nc.tensor.matmul(s4[:, j, :], M1_t[:, es:es + 128],
                                         xr1_nm[:, b, :], start=False, stop=False)
                        nc.tensor.matmul(s4[:, j, :], eT1_t[:, es:es + 128],
                                         We1p[:], start=False,
                                         stop=(j % 2 == 1 or j == nch - 1))
                    ls4 = sb.tile([128, 4, H1], dt.bfloat16, tag="ls4")
                    if P1 > 0:
                        nc.scalar.activation(ls4[:, :nch, 0:P1], s4[:, :nch, 0:P1],
                                             AF.Prelu, alpha=0.2)
                    if P1 < H1:
                        nc.scalar.activation(ls4[:, :nch, P1:H1], s4[:, :nch, P1:H1],
                                             AF.Prelu, scale=-0.2, alpha=5.0)
                    e4 = sb.tile([128, 4], dt.float32, tag="e4")
                    nc.vector.reduce_sum(e4[:, :nch], ls4[:, :nch, :],
                                         axis=mybir.AxisListType.X)
                    w4 = sb.tile([128, 4], dt.float32, tag="w4")
                    nc.scalar.activation(w4[:, :nch], e4[:, :nch], AF.Exp)
                    MwT = sb.tile([128, 4, 128], dt.bfloat16, tag="MwT")
                    for j in range(nch):
                        jg = o // 128 + 4 * g + j
                        nc.vector.tensor_scalar(
                            MwT[:, j, :], IOTA1[:],
                            dlc1_all[:, jg:jg + 1], w4[:, j:j + 1],
                            op0=Alu.is_equal, op1=Alu.mult)
                    for j in range(nch):
                        jj = 4 * g + j
                        nc.tensor.matmul(agg_den[:, 0:128], natE_t[:, jj, :],
                                         MwT[:, j, :],
                                         start=(jj == 0), stop=False)
                        nc.tensor.matmul(agg_den[:, 128:129], MwT[:, j, :],
                                         ones_col[:],
                                         start=False, stop=(jj == nchb - 1))
                # ---- block finalize: x1 = relu(agg/den + b1) ----------
                agsb = sb.tile([128, 128], dt.bfloat16, tag="agsb")
                nc.scalar.copy(agsb[:], agg_den[:, 0:128])
                den_c = sb.tile([128, 1], dt.float32, tag="denc")
                nc.vector.tensor_scalar(den_c[:], agg_den[:, 128:129], 1e-30, None,
                                        op0=Alu.max)
                rden = sb.tile([128, 1], dt.float32, tag="rden")
                nc.vector.reciprocal(rden[:], den_c[:])
                aggps = ps_fin.tile([128, H1], dt.float32, tag="fin")
                nc.tensor.matmul(aggps[:], agsb[:], Wl1p[:], start=True, stop=True)
                t1 = sb.tile([128, H1], dt.float32, tag="t1")
                nc.vector.tensor_scalar(t1[:], aggps[:], rden[:], None, op0=Alu.mult)
                t2 = sb.tile([128, H1], dt.float32, tag="t2")
                nc.vector.tensor_tensor(t2[:], t1[:], b1B[:], op=Alu.add)
                x1_nm = sb.tile([128, H1], dt.bfloat16, tag="x1nm")
                nc.scalar.activation(x1_nm[:], t2[:], AF.Relu)
                if dbg is not None:
                    nc.sync.dma_start(dbg["x1t2"][b * 128:(b + 1) * 128, :], t2[:])
                    nc.sync.dma_start(dbg["den1"][b, :], den_c[:, 0])
                # ---- layer-2 transforms for this block ----------------
                x1bT = sb.tile([128, H1 // 128, 128], dt.bfloat16, tag="x1bT")
                for hh in range(H1 // 128):
                    tp = ps_fin.tile([128, 128], dt.bfloat16, tag="tp")
                    nc.tensor.transpose(tp[:], x1_nm[:, hh * 128:(hh + 1) * 128],
                                        IDENT[:])
                    nc.scalar.copy(x1bT[:, hh, :], tp[:])
                xl2ps = ps_fin.tile([128, H2], dt.float32, tag="fin")
                for hh in range(H1 // 128):
                    nc.tensor.matmul(xl2ps[:], x1bT[:, hh, :],
                                     Wl2pp[:, hh * H2:(hh + 1) * H2],
                                     start=(hh == 0), stop=(hh == H1 // 128 - 1))
                nc.scalar.copy(xl2_nm[:, b, :], xl2ps[:])
                ci = int(chunk_of_blk[b])
                nc.sync.dma_start(
                    ag2_in[ci][(b - int(b0s[ci])) * 128:(b - int(b0s[ci]) + 1) * 128, :],
                    xl2_nm[:, b, :])
                xr2ps = ps_fin.tile([128, H2], dt.float32, tag="fin")
                for hh in range(H1 // 128):
                    nc.tensor.matmul(xr2ps[:], x1bT[:, hh, :],
                                     Wr2pp[:, hh * H2:(hh + 1) * H2],
                                     start=(hh == 0), stop=(hh == H1 // 128 - 1))
                nc.vector.tensor_tensor(xr2_nm[:, b, :], xr2ps[:], xr2bB[:], op=Alu.add)
                if dbg is not None:
                    dx = sb.tile([128, H2], dt.float32, tag="dbgx")
                    nc.vector.tensor_scalar(dx[:], xl2_nm[:, b, :], 0.0, None, op0=Alu.add)
                    nc.sync.dma_start(dbg["xl2"][b * 128:(b + 1) * 128, :], dx[:])
                    dx2 = sb.tile([128, H2], dt.float32, tag="dbgx2")
                    nc.vector.tensor_scalar(dx2[:], xr2_nm[:, b, :], 0.0, None, op0=Alu.add)
                    nc.sync.dma_start(dbg["xr2"][b * 128:(b + 1) * 128, :], dx2[:])
                if b == int(chunk_last_blk[ci]):
                    nc.gpsimd.collective_compute(
                        "AllGather", mybir.AluOpType.bypass,
                        replica_groups=[list(range(NCORE))],
                        ins=[ag2_in[ci][:]],
                        outs=[tbl2[int(cbase[ci]):int(cbase[ci]) + NCORE * int(rpc[ci]), :]])

        # ---------------- layer-2 edge phase + pooling ------------------
        pool_pp = ctx.enter_context(tc.tile_pool(name="poolps", bufs=1, space="PSUM"))
        pool_ps = pool_pp.tile([G, H2 + 4], dt.float32, tag="pool")
        pt_pool = ctx.enter_context(tc.tile_pool(name="ptsb", bufs=1))
        PT_sb = []
        for b in range(NBK):
            t = pt_pool.tile([128, G], dt.bfloat16, tag=f"pt{b}")
            nc.scalar.dma_start(t[:], I["PT"][b])
            PT_sb.append(t)

        with ExitStack() as ctx2:
            pre = ctx2.enter_context(tc.tile_pool(name="pre2", bufs=1))
            L2 = int(meta["L2"]); C2 = int(meta["C2"])
            idx_all = pre.tile([128, L2 // 16], dt.int16, tag="idxall")
            nc.scalar.dma_start(idx_all[:], I["idxs2"][:])
            dlc2_all = pre.tile([128, C2], dt.float32, tag="dlc2")
            nc.scalar.dma_start(dlc2_all[:], I["dlc2"][:])

            sbst = ctx2.enter_context(tc.tile_pool(name="sbst2", bufs=2))
            sbg = ctx2.enter_context(tc.tile_pool(name="sbg2", bufs=4))
            sb = ctx2.enter_context(tc.tile_pool(name="sb2", bufs=4))
            ps_s = ctx2.enter_context(tc.tile_pool(name="ps_s2", bufs=3, space="PSUM"))
            ps_agg = ctx2.enter_context(tc.tile_pool(name="ps_agg2", bufs=2, space="PSUM"))

            tlo = tbl2[0:HALF, :]
            thi = tbl2[HALF:NPAD2, :]

            for b in range(NBK):
                agg = ps_agg.tile([128, H2 + 4], dt.float32, tag="agg")
                first = True
                for h in range(2):
                    sl = int(seg2[b, h]); o = int(offs2[b, h])
                    if sl == 0:
                        continue
                    xlg3 = sbg.tile([128, sl // 128, H2], dt.bfloat16, tag="xlg")
                    nc.gpsimd.dma_gather(xlg3[:], thi if h else tlo,
                                         idx_all[:, o // 16:(o + sl) // 16], sl, sl, H2)
                    xlg = xlg3[:].rearrange("p a b -> p (a b)")
                    if dbg is not None and b == 0 and h == 0:
                        dxg = sb.tile([128, (sl // 128) * H2], dt.float32, tag="dbgxg")
                        nc.vector.tensor_scalar(dxg[:], xlg, 0.0, None, op0=Alu.add)
                        nc.sync.dma_start(dbg["xlg0"][:], dxg[:])
                    eT2_t = sbst.tile([ED, sl], dt.bfloat16, tag="eT2")
                    nc.sync.dma_start(eT2_t[:], I["eT2"][:, o:o + sl])
                    M2_t = sbst.tile([128, sl], dt.bfloat16, tag="M2")
                    nc.sync.dma_start(M2_t[:], I["M2"][:, o:o + sl])
                    for po in range(0, sl, 512):
                        pl = min(512, sl - po)
                        nch = pl // 128
                        jj0 = (o + po) // 128
                        s4 = ps_s.tile([128, 4, H2], dt.float32, tag="s4")
                        nc.tensor.matmul(
                            s4[:, 0:nch, :], IDENT[:],
                            xlg[:, (po // 128) * H2:(po // 128 + nch) * H2],
                            start=True, stop=False)
                        for j in range(nch):
                            es = po + j * 128
                            nc.tensor.matmul(s4[:, j, :], eT2_t[:, es:es + 128],
                                             We2p[:], start=False, stop=False)
                            nc.tensor.matmul(s4[:, j, :], M2_t[:, es:es + 128],
                                             xr2_nm[:, b, :], start=False,
                                             stop=(j == nch - 1))
                        ls4 = sb.tile([128, 4, H2], dt.bfloat16, tag="ls4")
                        if P2 > 0:
                            nc.scalar.activation(ls4[:, :nch, 0:P2], s4[:, :nch, 0:P2],
                                                 AF.Prelu, alpha=0.2)
                        if P2 < H2:
                            nc.scalar.activation(ls4[:, :nch, P2:H2], s4[:, :nch, P2:H2],
                                                 AF.Prelu, scale=-0.2, alpha=5.0)
                        e4 = sb.tile([128, 4], dt.float32, tag="e4")
                        nc.vector.reduce_sum(e4[:, :nch], ls4[:, :nch, :],
                                             axis=mybir.AxisListType.X)
                        w4 = sb.tile([128, 4], dt.float32, tag="w4")
                        nc.scalar.activation(w4[:, :nch], e4[:, :nch], AF.Exp)
                        MwT = sb.tile([128, 4, 128], dt.bfloat16, tag="MwT")
                        for j in range(nch):
                            nc.vector.tensor_scalar(
                                MwT[:, j, :], IOTA1[:],
                                dlc2_all[:, jj0 + j:jj0 + j + 1], w4[:, j:j + 1],
                                op0=Alu.is_equal, op1=Alu.mult)
                        for j in range(nch):
                            nc.tensor.matmul(
                                agg[:, 0:H2], MwT[:, j, :],
                                xlg[:, (po // 128 + j) * H2:(po // 128 + j + 1) * H2],
                                start=first, stop=False)
                            nc.tensor.matmul(agg[:, H2:H2 + 1], MwT[:, j, :],
                                             ones_col[:], start=False, stop=False)
                            first = False
                # ---- self chunk --------------------------------------
                s_s = ps_s.tile([128, 4, H2], dt.float32, tag="s4")
                nc.tensor.matmul(s_s[:, 0, :], laT2[:, b * 128:(b + 1) * 128],
                                 We2p[:], start=True, stop=False)
                nc.tensor.matmul(s_s[:, 0, :], IDENT[:], xr2_nm[:, b, :],
                                 start=False, stop=False)
                nc.tensor.matmul(s_s[:, 0, :], IDENT[:], xl2_nm[:, b, :],
                                 start=False, stop=True)
                ls_s = sb.tile([128, 4, H2], dt.bfloat16, tag="ls4")
                if P2 > 0:
                    nc.scalar.activation(ls_s[:, 0, 0:P2], s_s[:, 0, 0:P2],
                                         AF.Prelu, alpha=0.2)
                if P2 < H2:
                    nc.scalar.activation(ls_s[:, 0, P2:H2], s_s[:, 0, P2:H2],
                                         AF.Prelu, scale=-0.2, alpha=5.0)
                es_ = sb.tile([128, 1], dt.float32, tag="es")
                nc.vector.reduce_sum(es_[:], ls_s[:, 0:1, :], axis=mybir.AxisListType.X)
                ws = sb.tile([128, 1], dt.float32, tag="ws")
                nc.scalar.activation(ws[:], es_[:], AF.Exp)
                diagw = sb.tile([128, 128], dt.bfloat16, tag="diagw")
                nc.vector.tensor_scalar(diagw[:], IDENT[:], ws[:], None, op0=Alu.mult)
                nc.tensor.matmul(agg[:, 0:H2], diagw[:], xl2_nm[:, b, :],
                                 start=False, stop=False)
                nc.tensor.matmul(agg[:, H2:H2 + 1], diagw[:], ones_col[:],
                                 start=False, stop=True)
                # ---- finalize block: x2 = relu(agg/den + b2) ----------
                rden = sb.tile([128, 1], dt.float32, tag="rden")
                nc.vector.reciprocal(rden[:], agg[:, H2:H2 + 1])
                t1 = sb.tile([128, H2], dt.float32, tag="t1")
                nc.vector.tensor_scalar(t1[:], agg[:, 0:H2], rden[:], None, op0=Alu.mult)
                t2 = sb.tile([128, H2], dt.float32, tag="t2")
                nc.vector.tensor_tensor(t2[:], t1[:], b2B[:], op=Alu.add)
                x2_nm = sb.tile([128, H2], dt.bfloat16, tag="x2nm")
                nc.scalar.activation(x2_nm[:], t2[:], AF.Relu)
                nc.tensor.matmul(pool_ps[:, 0:H2], PT_sb[b][:], x2_nm[:],
                                 start=(b == 0), stop=(b == NBK - 1))

        if dbg is not None:
            with tc.tile_pool(name="dbgt", bufs=2) as dbgt:
                for r0 in range(0, NPAD2, 128):
                    tt = dbgt.tile([128, H2], dt.bfloat16, tag="t2d")
                    nc.scalar.dma_start(tt[:], tbl2[r0:r0 + 128, :])
                    nc.scalar.dma_start(dbg["tbl2"][r0:r0 + 128, :], tt[:])

        # ---------------- head -----------------------------------------
        with tc.tile_pool(name="hsb", bufs=2) as hsb, \
             tc.tile_pool(name="hps", bufs=2, space="PSUM") as hps:
            psb = hsb.tile([G, H2], dt.float32, tag="poolsb")
            nc.scalar.copy(psb[:], pool_ps[:, 0:H2])
            nc.sync.dma_start(pool_in[:], psb[:])
            nc.gpsimd.collective_compute(
                "AllReduce", mybir.AluOpType.add,
                replica_groups=[list(range(NCORE))],
                ins=[pool_in[:]], outs=[pool_out[:]])
            pooled = hsb.tile([G, H2], dt.float32, tag="pooled")
            nc.sync.dma_start(pooled[:], pool_out[:])
            pooled_T_ps = hps.tile([H2, G], dt.float32, tag="pooledT")
            nc.tensor.transpose(pooled_T_ps[:], pooled[:], IDENT32[0:G, 0:G])
            pooled_T = hsb.tile([H2, G], dt.float32, tag="pooledTsb")
            nc.scalar.copy(pooled_T[:], pooled_T_ps[:])
            Wd1sb = hsb.tile([H2, HD], dt.float32, tag="wd1")
            nc.sync.dma_start(Wd1sb[:], I["Wd1u"][:])
            h1ps = hps.tile([HD, G], dt.float32, tag="h1")
            nc.tensor.matmul(h1ps[:], Wd1sb[:], pooled_T[:], start=True, stop=True)
            hscale = hsb.tile([HD, 1], dt.float32, tag="hscale")
            nc.sync.dma_start(hscale[:], I["head_scale"][:])
            hbias = hsb.tile([HD, 1], dt.float32, tag="hbias")
            nc.sync.dma_start(hbias[:], I["head_bias"][:])
            th = hsb.tile([HD, G], dt.float32, tag="th")
            nc.scalar.activation(th[:], h1ps[:], AF.Prelu, bias=hbias[:],
                                 scale=hscale[:], alpha=0.1)
            Wd2sb = hsb.tile([HD, OUT], dt.float32, tag="wd2")
            nc.sync.dma_start(Wd2sb[:], I["Wd2"][:])
            ops = hps.tile([OUT, G], dt.float32, tag="ops")
            nc.tensor.matmul(ops[:], Wd2sb[:], th[:], start=True, stop=True)
            bd2sb = hsb.tile([OUT, 1], dt.float32, tag="bd2sb")
            nc.sync.dma_start(bd2sb[:], I["bd2"][:])
            osb = hsb.tile([OUT, G], dt.float32, tag="osb")
            nc.vector.tensor_scalar(osb[:], ops[:], bd2sb[:], None, op0=Alu.add)
            nc.sync.dma_start(out_t[:], osb[:])


def _kernel(inputs, cfg, runner=None, trace=False):
    com, percore, meta = host_prep(inputs, cfg)
    nc = build_program(meta, com, percore[0])
    in_maps = [dict(com, **pc) for pc in percore]
    if runner is None:
        from concourse.bass_utils import run_bass_kernel_spmd
        res = run_bass_kernel_spmd(nc, in_maps, list(range(cfg["NC"])), trace=trace)
        out = np.asarray(res.results[0]["out"])
        return out.T.copy().astype(np.float32), res
    return runner(nc, in_maps)


def kernel(**inputs):
    out, _ = _kernel(inputs, DEFAULT_CFG)
    return out
